# revision 1
# baseline (speedup 1.0000x reference)
"""Causal MQA self-attention (RoPE + RMS-norm on q/k) on 8 TRN2 NeuronCores.

Sharding: core c -> (batch b = c//4, head-group g = c%4 of 4 heads).
Each core computes, for its batch and its 4 heads:
  q/k/v projections -> RoPE -> RMS-norm -> causal attention -> partial
  output projection out_part = attn_out_g @ wo[:, g].T  (shape [S, HID]).
Host sums the 4 per-group partials of each batch (row-parallel matmul
unshard) and stacks the 2 batches.

PE-facing tensors are bf16 (fp32 PSUM accumulation); softmax runs
without max-subtraction (post-RMS-norm scores <= sqrt(D) ~ 11.3, exp
in range). Attention output is produced TRANSPOSED ([d, q] = v.T @ p.T
via 512-wide moving matmuls) so it feeds the output projection without
extra transposes; the softmax denominator comes from a [1,512] ones-row
matmul accumulated in PSUM, inverted and partition-broadcast on GpSimd.
"""

import ml_dtypes
import numpy as np

import concourse.bass as bass
import concourse.mybir as mybir
import concourse.tile as tile
from concourse import bacc
from concourse.bass_utils import run_bass_kernel_spmd
from concourse.masks import make_identity

# problem dims (hardcoded per contract)
B, S, HID, H, D = 2, 2048, 2048, 16, 128
NCORES = 8
GROUPS = 4              # head-groups = cores per batch
HG = H // GROUPS        # heads per core
DG = HG * D             # 512 projected q dims per core
NT = S // 128           # 16 sequence tiles
HT = HID // 128         # 16 hidden tiles
NQC = 4                 # q chunks of 512 columns
EPS = 1.1920928955078125e-07
ISD = 1.0 / float(np.sqrt(D))

f32 = mybir.dt.float32
bf16 = mybir.dt.bfloat16

TRACE = False           # test harness may flip this for NTFF profiling
LAST = {}               # last BassKernelResults, for the test harness
PH1_TILES = NT          # bisect knob
PH2_CHUNKS = NQC        # bisect knob

_compiled = None


def _emit(nc, xT, wqT, wkvT, woT, csx, snx, cmw, out):
    add = mybir.AluOpType.add
    Sqrt = mybir.ActivationFunctionType.Sqrt
    Exp = mybir.ActivationFunctionType.Exp

    with tile.TileContext(nc) as tc:
        with (
            tc.tile_pool(name="consts", bufs=1) as consts,
            tc.tile_pool(name="bigp", bufs=1) as bigp,
            tc.tile_pool(name="xsp", bufs=3) as xsp,
            tc.tile_pool(name="csp", bufs=2) as csp,
            tc.tile_pool(name="rsp", bufs=3) as rsp,
            tc.tile_pool(name="smp", bufs=4) as smp,
            tc.tile_pool(name="qnp", bufs=4) as qnp,
            tc.tile_pool(name="ptp", bufs=10) as ptp,
            tc.tile_pool(name="otp", bufs=2) as otp,
            tc.tile_pool(name="ocp", bufs=4) as ocp,
            tc.tile_pool(name="pA", bufs=3, space="PSUM") as pA,
            tc.tile_pool(name="pX", bufs=2, space="PSUM") as pX,
        ):
            # ---- constants ----
            ident = consts.tile([128, 128], bf16)
            make_identity(nc, ident)
            eps_t = consts.tile([128, 1], f32)
            nc.vector.memset(eps_t, EPS)
            cmw_sb = consts.tile([128, NQC, DG], bf16)  # wide causal masks
            nc.sync.dma_start(cmw_sb, cmw.rearrange("p (k q) -> p k q", k=NQC))

            # ---- resident weights / activations ----
            wq_sb = bigp.tile([128, HT, DG], bf16, tag="wq")
            nc.sync.dma_start(wq_sb, wqT.rearrange("(t p) d -> p t d", p=128))
            wkv_sb = bigp.tile([128, HT, 2 * D], bf16, tag="wkv")
            nc.sync.dma_start(wkv_sb, wkvT.rearrange("(t p) d -> p t d", p=128))
            wo_sb = bigp.tile([128, HG, HID], bf16, tag="wo")
            nc.sync.dma_start(wo_sb, woT.rearrange("(h p) n -> p h n", p=128))

            qT_all = bigp.tile([128, HG, S], bf16, tag="qT")   # [d, h, s]
            kT_sb = bigp.tile([128, S], bf16, tag="kT")        # [d, s]
            vvb = bigp.tile([128, NT, 132], bf16, tag="vv")    # [s%128, s//128, d|ones]
            nc.vector.memset(vvb[:, :, 128:132], 1.0)

            xTr = xT.rearrange("(t p) s -> p t s", p=128)

            def bcast4(src2d, st):
                base = src2d[st * 128:(st + 1) * 128, :]
                return bass.AP(
                    tensor=base.tensor,
                    offset=base.offset,
                    ap=[base.ap[0], [0, HG], base.ap[1]],
                )

            def emit_st(st):
                xs0 = xsp.tile([128, HT // 2, 128], bf16, tag="xs")
                nc.sync.dma_start(xs0, xTr[:, 0:HT // 2, st * 128:(st + 1) * 128])
                xs1 = xsp.tile([128, HT // 2, 128], bf16, tag="xs")
                nc.sync.dma_start(xs1, xTr[:, HT // 2:HT, st * 128:(st + 1) * 128])
                xhalves = (xs0, xs1)

                cs_t = csp.tile([128, HG, 128], f32, tag="cs")
                nc.gpsimd.dma_start(cs_t, bcast4(csx, st))
                sn_t = csp.tile([128, HG, 128], f32, tag="sn")
                nc.gpsimd.dma_start(sn_t, bcast4(snx, st))

                qp = pA.tile([128, 2, DG], f32, tag="A")
                for t in range(HT):
                    nc.tensor.matmul(
                        qp[:, 0, :], lhsT=xhalves[t // 8][:, t % 8, :],
                        rhs=wq_sb[:, t, :], start=(t == 0), stop=(t == HT - 1),
                    )
                kvp = pA.tile([128, 2, DG], f32, tag="A")
                for t in range(HT):
                    nc.tensor.matmul(
                        kvp[:, 0, 0:2 * D], lhsT=xhalves[t // 8][:, t % 8, :],
                        rhs=wkv_sb[:, t, :], start=(t == 0), stop=(t == HT - 1),
                    )

                # stage kv psum -> SBUF on ACT, then v -> bf16 tiles
                kvs = rsp.tile([128, 2 * D], f32, tag="kvs")
                nc.scalar.copy(kvs, kvp[:, 0, 0:2 * D])
                nc.vector.tensor_copy(vvb[:, st, 0:128], kvs[:, D:2 * D])

                # ---- RoPE + RMS-norm for 4 q heads, batched ----
                # stage psum -> SBUF on ACT (frees the PSUM slot early and
                # lets the DVE ops run in SBUF-only 2x mode)
                qs = rsp.tile([128, DG], f32, tag="qs")
                nc.scalar.copy(qs, qp[:, 0, :])
                q3 = qs.rearrange("p (h d) -> p h d", h=HG)
                q4 = qs.rearrange("p (h t d) -> p h t d", h=HG, t=2)
                rot = rsp.tile([128, DG], f32, tag="rot")
                r4 = rot.rearrange("p (h t d) -> p h t d", h=HG, t=2)
                r3 = rot.rearrange("p (h d) -> p h d", h=HG)
                nc.scalar.copy(r4[:, :, 0, :], q4[:, :, 1, :])
                nc.scalar.mul(r4[:, :, 1, :], q4[:, :, 0, :], -1.0)
                t1 = rsp.tile([128, DG], f32, tag="t1")
                t3 = t1.rearrange("p (h d) -> p h d", h=HG)
                nc.vector.tensor_mul(t3, q3, cs_t)
                nc.vector.tensor_mul(r3, r3, sn_t)
                nc.vector.tensor_add(t1, t1, rot)          # t1 = roped q
                ms4 = smp.tile([128, HG], f32, tag="ms4")
                nc.vector.tensor_mul(rot, t1, t1)          # rot dead; reuse as q^2
                nc.vector.tensor_reduce(
                    ms4, rot.rearrange("p (h d) -> p h d", h=HG),
                    axis=mybir.AxisListType.X, op=add)
                srt4 = smp.tile([128, HG], f32, tag="srt4")
                nc.scalar.activation(out=srt4, in_=ms4, func=Sqrt,
                                     bias=eps_t[:, 0:1], scale=1.0 / D)
                nc.vector.reciprocal(srt4, srt4)
                for h in range(HG):
                    qn = qnp.tile([128, 128], bf16, tag="qn")
                    nc.vector.tensor_scalar_mul(
                        qn, t1[:, h * 128:(h + 1) * 128], srt4[:, h:h + 1])
                    tp = pX.tile([128, DG], f32, tag="X")
                    nc.tensor.transpose(tp[:, 0:64].bitcast(bf16), qn, ident)
                    nc.scalar.copy(
                        qT_all[:, h, st * 128:(st + 1) * 128],
                        tp[:, 0:64].bitcast(bf16))

                # ---- RoPE + RMS-norm for k (single kv head) ----
                kk = kvs[:, 0:D]
                k2 = kk.rearrange("p (t d) -> p t d", t=2)
                krot = rsp.tile([128, 128], f32, tag="krot")
                kr2 = krot.rearrange("p (t d) -> p t d", t=2)
                nc.scalar.copy(kr2[:, 0, :], k2[:, 1, :])
                nc.scalar.mul(kr2[:, 1, :], k2[:, 0, :], -1.0)
                kt1 = rsp.tile([128, 128], f32, tag="kt1")
                nc.vector.tensor_mul(kt1, kk, cs_t[:, 0, :])
                nc.vector.tensor_mul(krot, krot, sn_t[:, 0, :])
                nc.vector.tensor_add(kt1, kt1, krot)
                msk = smp.tile([128, 1], f32, tag="msk")
                nc.vector.tensor_mul(krot, kt1, kt1)       # krot dead; reuse as k^2
                nc.vector.tensor_reduce(msk, krot, axis=mybir.AxisListType.X, op=add)
                srtk = smp.tile([128, 1], f32, tag="srtk")
                nc.scalar.activation(out=srtk, in_=msk, func=Sqrt,
                                     bias=eps_t[:, 0:1], scale=1.0 / D)
                nc.vector.reciprocal(srtk, srtk)
                kn = qnp.tile([128, 128], bf16, tag="kn")
                nc.vector.tensor_scalar_mul(kn, kt1, srtk)
                tp = pX.tile([128, DG], f32, tag="X")
                nc.tensor.transpose(tp[:, 0:64].bitcast(bf16), kn, ident)
                nc.scalar.copy(
                    kT_sb[:, st * 128:(st + 1) * 128],
                    tp[:, 0:64].bitcast(bf16))

            def emit_qc(qc):
                otile = otp.tile([128, HG, DG], bf16, tag="ot")  # [d, h, q]
                nkt = 4 * (qc + 1)
                for h in range(HG):
                    qrhs = qT_all[:, h, qc * DG:(qc + 1) * DG]
                    pts = []
                    for j2 in range(0, nkt, 2):
                        sp = pA.tile([128, 2, DG], f32, tag="A")
                        for j in range(2):
                            kt = j2 + j
                            nc.tensor.matmul(
                                sp[:, j, :],
                                lhsT=kT_sb[:, kt * 128:(kt + 1) * 128],
                                rhs=qrhs, start=True, stop=True)
                        pt = ptp.tile([128, 2, DG], bf16, tag="pt")
                        # ragged exp: skip fully-masked (k > q) spans of
                        # diagonal-range k tiles; those pT regions are never
                        # read by the causal PV loop below.
                        for j in range(2):
                            kt = j2 + j
                            qoff = max(0, (kt - 4 * qc)) * 128
                            if qoff >= DG:
                                continue
                            nc.scalar.activation(
                                out=pt[:, j, qoff:DG], in_=sp[:, j, qoff:DG],
                                func=Exp, scale=ISD)
                        pts.append(pt)
                    # causal masking of the 4 diagonal k tiles (tri block)
                    for qtl in range(4):
                        kt = 4 * qc + qtl
                        sl = pts[kt // 2][:, kt % 2, qtl * 128:(qtl + 1) * 128]
                        nc.vector.tensor_mul(sl, sl, cmw_sb[:, qtl, qtl * 128:(qtl + 1) * 128])
                    # probs @ [v | ones] per q tile
                    for qtl in range(4):
                        qt = 4 * qc + qtl
                        op = pX.tile([128, DG], f32, tag="X")
                        for kt in range(qt + 1):
                            nc.tensor.matmul(
                                op[:, 0:129],
                                lhsT=pts[kt // 2][:, kt % 2, qtl * 128:(qtl + 1) * 128],
                                rhs=vvb[:, kt, 0:129],
                                start=(kt == 0), stop=(kt == qt))
                        rc = smp.tile([128, 1], f32, tag="rc")
                        nc.vector.reciprocal(rc, op[:, 128:129])
                        on = qnp.tile([128, 128], bf16, tag="on")
                        nc.vector.tensor_scalar_mul(on, op[:, 0:128], rc)
                        tp = pX.tile([128, DG], f32, tag="X")
                        nc.tensor.transpose(tp[:, 0:64].bitcast(bf16), on, ident)
                        nc.vector.tensor_copy(
                            otile[:, h, qtl * 128:(qtl + 1) * 128],
                            tp[:, 0:64].bitcast(bf16))
                # output projection for this chunk's 4 row tiles
                for stl in range(4):
                    srow = (4 * qc + stl) * 128
                    for cc in range(4):
                        wop = pA.tile([128, 2, DG], f32, tag="A")
                        for h2 in range(HG):
                            nc.tensor.matmul(
                                wop[:, 0, :],
                                lhsT=otile[:, h2, stl * 128:(stl + 1) * 128],
                                rhs=wo_sb[:, h2, cc * DG:(cc + 1) * DG],
                                start=(h2 == 0), stop=(h2 == HG - 1))
                        oc = ocp.tile([128, DG], f32, tag="oc")
                        if cc % 2 == 0:
                            nc.vector.tensor_copy(oc, wop[:, 0, :])
                        else:
                            nc.scalar.copy(oc, wop[:, 0, :])
                        nc.sync.dma_start(
                            out[srow:srow + 128, cc * DG:(cc + 1) * DG], oc)



            # interleave: chunk qc only needs s-tiles <= 4*qc+3, so emit its
            # attention right after those tiles -- exp overlaps later proj work
            for st in range(PH1_TILES):
                emit_st(st)
                if st % 4 == 3 and (st // 4) < PH2_CHUNKS:
                    emit_qc(st // 4)


def _build():
    nc = bacc.Bacc("TRN2", target_bir_lowering=False, debug=False,
                   num_devices=NCORES)
    xT = nc.dram_tensor("xT", [HID, S], bf16, kind="ExternalInput").ap()
    wqT = nc.dram_tensor("wqT", [HID, DG], bf16, kind="ExternalInput").ap()
    wkvT = nc.dram_tensor("wkvT", [HID, 2 * D], bf16, kind="ExternalInput").ap()
    woT = nc.dram_tensor("woT", [DG, HID], bf16, kind="ExternalInput").ap()
    csx = nc.dram_tensor("csx", [S, 128], f32, kind="ExternalInput").ap()
    snx = nc.dram_tensor("snx", [S, 128], f32, kind="ExternalInput").ap()
    cmw = nc.dram_tensor("cmw", [128, NQC * DG], bf16, kind="ExternalInput").ap()
    out = nc.dram_tensor("out", [S, HID], f32, kind="ExternalOutput").ap()
    _emit(nc, xT, wqT, wkvT, woT, csx, snx, cmw, out)
    nc.compile()
    return nc


def _get_compiled():
    global _compiled
    if _compiled is None:
        _compiled = _build()
    return _compiled


def _causal_masks():
    """cmw[k, ktl, q]: per diagonal-position wide mask over a 512-q chunk."""
    m = np.zeros((128, NQC, DG), np.float32)
    tri = np.triu(np.ones((128, 128), np.float32))  # 1 where k <= q
    for ktl in range(4):
        for qt in range(4):
            blk = m[:, ktl, qt * 128:(qt + 1) * 128]
            if qt > ktl:
                blk[:] = 1.0
            elif qt == ktl:
                blk[:] = tri
    return np.ascontiguousarray(
        m.reshape(128, NQC * DG).astype(ml_dtypes.bfloat16))


def kernel(x, cos, sin, wq, wk, wv, wo):
    nc = _get_compiled()
    x = np.asarray(x, np.float32)
    cos = np.asarray(cos, np.float32)
    sin = np.asarray(sin, np.float32)
    wq = np.asarray(wq, np.float32)
    wk = np.asarray(wk, np.float32)
    wv = np.asarray(wv, np.float32)
    wo = np.asarray(wo, np.float32)

    bf = ml_dtypes.bfloat16
    wkvT = np.ascontiguousarray(np.concatenate([wk, wv], 0).T.astype(bf))
    csx = np.ascontiguousarray(np.concatenate([cos, cos], 1))
    snx = np.ascontiguousarray(np.concatenate([sin, sin], 1))
    cmw = _causal_masks()
    xTs = [np.ascontiguousarray(x[b].T.astype(bf)) for b in range(B)]
    wqTs = [np.ascontiguousarray(wq[g * DG:(g + 1) * DG].T.astype(bf))
            for g in range(GROUPS)]
    woTs = [np.ascontiguousarray(wo[:, g * DG:(g + 1) * DG].T.astype(bf))
            for g in range(GROUPS)]

    in_maps = []
    for c in range(NCORES):
        b, g = divmod(c, GROUPS)
        in_maps.append({
            "xT": xTs[b], "wqT": wqTs[g], "wkvT": wkvT, "woT": woTs[g],
            "csx": csx, "snx": snx, "cmw": cmw,
        })
    res = run_bass_kernel_spmd(nc, in_maps, list(range(NCORES)), trace=TRACE)
    LAST["res"] = res
    outs = [r["out"] for r in res.results]
    final = np.empty((B, S, HID), np.float32)
    for b in range(B):
        final[b] = (outs[GROUPS * b] + outs[GROUPS * b + 1]
                    + outs[GROUPS * b + 2] + outs[GROUPS * b + 3])
    return final



# revision 5
# speedup vs baseline: 1.1220x; 1.1220x over previous
"""Causal MQA self-attention (RoPE + RMS-norm on q/k) on 8 TRN2 NeuronCores.

Sharding: core c -> (batch b = c//4, head-group g = c%4 of 4 heads).
Each core computes, for its batch and its 4 heads:
  q/k/v projections -> RoPE -> RMS-norm -> causal attention -> partial
  output projection out_part = attn_out_g @ wo[:, g].T  (shape [S, HID]).
Host sums the 4 per-group partials of each batch (row-parallel matmul
unshard) and stacks the 2 batches.

v2 design notes:
- All transposes run on the DMA xbar (dma_start_transpose), batched 5
  tiles at a time ([s,640] -> [d, 5, 128]), not on the PE.
- RoPE + RMS-norm run in bf16 on the DVE in 2x mode; sin is host-prepped
  as [sin, -sin] so rotate-half is two strided multiplies (no copies).
- Scores diagonal blocks use narrowed rhs (ragged causal trimming); exp
  on fully-causal pairs is fused into [128,1024] instructions.
- Softmax denominator comes from a 129th ones-column on the PV matmul;
  normalization is recip + tensor_scalar (also the bf16 cast).
- Output projection partials are written bf16; host accumulates in f32.
"""

import ml_dtypes
import numpy as np

import concourse.bass as bass
import concourse.mybir as mybir
import concourse.tile as tile
from concourse import bacc
from concourse.bass_utils import run_bass_kernel_spmd

# problem dims (hardcoded per contract)
B, S, HID, H, D = 2, 2048, 2048, 16, 128
NCORES = 8
GROUPS = 4              # head-groups = cores per batch
HG = H // GROUPS        # heads per core
DG = HG * D             # 512 projected q dims per core
NT = S // 128           # 16 sequence tiles
HT = HID // 128         # 16 hidden tiles
NQC = 4                 # q chunks of 512 columns
EPS = 1.1920928955078125e-07
ISD = 1.0 / float(np.sqrt(D))

f32 = mybir.dt.float32
bf16 = mybir.dt.bfloat16

TRACE = False           # test harness may flip this for NTFF profiling
LAST = {}               # last BassKernelResults, for the test harness
PH1_TILES = NT          # bisect knob
PH2_CHUNKS = NQC        # bisect knob

_compiled = None


def _bcast_h(src, nh):
    """AP view of a [128, N] tile as [128, nh, N] with stride-0 head dim."""
    return bass.AP(tensor=src.tensor, offset=src.offset,
                   ap=[src.ap[0], [0, nh], src.ap[-1]])


def _emit(nc, xT, wqT, wkvT, woT, csx, snx, tri, out):
    add = mybir.AluOpType.add
    Sqrt = mybir.ActivationFunctionType.Sqrt
    Exp = mybir.ActivationFunctionType.Exp

    with tile.TileContext(nc) as tc:
        with (
            tc.tile_pool(name="consts", bufs=1) as consts,
            tc.tile_pool(name="bigp", bufs=1) as bigp,
            tc.tile_pool(name="xsp", bufs=4) as xsp,
            tc.tile_pool(name="rsp", bufs=3) as rsp,
            tc.tile_pool(name="smp", bufs=4) as smp,
            tc.tile_pool(name="qnp", bufs=3) as qnp,
            tc.tile_pool(name="ptp", bufs=10) as ptp,
            tc.tile_pool(name="onp", bufs=8) as onp,
            tc.tile_pool(name="otp", bufs=2) as otp,
            tc.tile_pool(name="ocp", bufs=4) as ocp,
            tc.tile_pool(name="pA", bufs=3, space="PSUM") as pA,
            tc.tile_pool(name="pX", bufs=2, space="PSUM") as pX,
        ):
            # ---- constants ----
            eps_t = consts.tile([128, 1], f32)
            nc.vector.memset(eps_t, EPS)
            tri_sb = consts.tile([128, 128], bf16)
            nc.sync.dma_start(tri_sb, tri)

            # ---- resident weights / activations ----
            wq_sb = bigp.tile([128, HT, DG], bf16, tag="wq")
            nc.sync.dma_start(wq_sb, wqT.rearrange("(t p) d -> p t d", p=128))
            wkv_sb = bigp.tile([128, HT, 2 * D], bf16, tag="wkv")
            nc.sync.dma_start(wkv_sb, wkvT.rearrange("(t p) d -> p t d", p=128))
            wo_sb = bigp.tile([128, HG, HID], bf16, tag="wo")
            nc.sync.dma_start(wo_sb, woT.rearrange("(h p) n -> p h n", p=128))
            cs_sb = bigp.tile([128, NT, 128], bf16, tag="cs")
            nc.sync.dma_start(cs_sb, csx.rearrange("(t p) d -> p t d", p=128))
            sn_sb = bigp.tile([128, NT, 128], bf16, tag="sn")
            nc.sync.dma_start(sn_sb, snx.rearrange("(t p) d -> p t d", p=128))

            # qkT[d, j, s]: j<4 = transposed-normed q heads, j=4 = k
            qkT = bigp.tile([128, 5, S], bf16, tag="qkT")
            vvb = bigp.tile([128, NT, 132], bf16, tag="vv")   # [s%128, kt, d|ones]
            nc.vector.memset(vvb[:, :, 128:132], 1.0)

            xTr = xT.rearrange("(t p) s -> p t s", p=128)

            def emit_st(st):
                sl = slice(st * 128, (st + 1) * 128)
                xs0 = xsp.tile([128, HT // 2, 128], bf16, tag="xs")
                nc.sync.dma_start(xs0, xTr[:, 0:HT // 2, sl])
                xs1 = xsp.tile([128, HT // 2, 128], bf16, tag="xs")
                nc.sync.dma_start(xs1, xTr[:, HT // 2:HT, sl])
                xhalves = (xs0, xs1)

                qkv = pA.tile([128, 2, DG], f32, tag="A")
                for t in range(HT):
                    nc.tensor.matmul(
                        qkv[:, 0, :], lhsT=xhalves[t // 8][:, t % 8, :],
                        rhs=wq_sb[:, t, :], start=(t == 0), stop=(t == HT - 1))
                for t in range(HT):
                    nc.tensor.matmul(
                        qkv[:, 1, 0:2 * D], lhsT=xhalves[t // 8][:, t % 8, :],
                        rhs=wkv_sb[:, t, :], start=(t == 0), stop=(t == HT - 1))

                # stage to bf16 SBUF: [0:512]=q heads, [512:640]=k, [640:768]=v
                qkv8 = rsp.tile([128, 768], bf16, tag="qkv8")
                nc.scalar.copy(qkv8[:, 0:DG], qkv[:, 0, :])
                nc.scalar.copy(qkv8[:, DG:DG + 2 * D], qkv[:, 1, 0:2 * D])
                nc.vector.tensor_copy(vvb[:, st, 0:128], qkv8[:, 640:768])

                # ---- RoPE (bf16, DVE 2x) over q heads + k as 5 groups ----
                qk5 = qkv8[:, 0:640].rearrange("p (h t d) -> p h t d", h=5, t=2)
                rot = rsp.tile([128, 640], bf16, tag="rot")
                r5 = rot.rearrange("p (h t d) -> p h t d", h=5, t=2)
                sn_t = sn_sb[:, st, :]
                nc.vector.tensor_mul(r5[:, :, 0, :], qk5[:, :, 1, :],
                                     _bcast_h(sn_t[0:128, 0:64], 5))
                nc.vector.tensor_mul(r5[:, :, 1, :], qk5[:, :, 0, :],
                                     _bcast_h(sn_t[0:128, 64:128], 5))
                t1 = rsp.tile([128, 640], bf16, tag="t1")
                t5 = t1.rearrange("p (h d) -> p h d", h=5)
                nc.vector.tensor_mul(t5, qkv8[:, 0:640].rearrange(
                    "p (h d) -> p h d", h=5), _bcast_h(cs_sb[:, st, :], 5))
                nc.vector.tensor_add(t1, t1, rot)          # t1 = roped qk

                # ---- RMS-norm factors for the 5 groups ----
                nc.vector.tensor_mul(rot, t1, t1)          # rot dead; reuse as sq
                ms5 = smp.tile([128, 5], bf16, tag="ms5")
                with nc.allow_low_precision(reason="rms stats tolerate bf16"):
                    nc.vector.tensor_reduce(
                        ms5, rot.rearrange("p (h d) -> p h d", h=5),
                        axis=mybir.AxisListType.X, op=add)
                srt = smp.tile([128, 5], f32, tag="srt")
                nc.scalar.activation(out=srt, in_=ms5, func=Sqrt,
                                     bias=eps_t[:, 0:1], scale=1.0 / D)
                srtb = smp.tile([128, 5], bf16, tag="srtb")
                with nc.allow_low_precision(reason="rms scale tolerates bf16"):
                    nc.vector.reciprocal(srtb, srt)
                qkn = qnp.tile([128, 640], bf16, tag="qkn")
                nc.vector.tensor_mul(
                    qkn.rearrange("p (h d) -> p h d", h=5),
                    t1.rearrange("p (h d) -> p h d", h=5),
                    bass.AP(tensor=srtb.tensor, offset=srtb.offset,
                            ap=[srtb.ap[0], [1, 5], [0, 128]]))
                # batched xbar transpose: [s, 5*128] -> qkT[d, 5, s-slice]
                nc.sync.dma_start_transpose(qkT[:, :, sl], qkn)

            def emit_qc(qc):
                nkt = 4 * (qc + 1)
                on_tiles = []
                for _qtl in range(4):
                    on_t = onp.tile([128, HG, 128], bf16, tag="on")
                    on_tiles.append(on_t)
                for h in range(HG):
                    pts = []
                    for j2 in range(nkt // 2):
                        sp = pA.tile([128, 2, DG], f32, tag="A")
                        for j in range(2):
                            kt = 2 * j2 + j
                            qoff = max(0, (kt - 4 * qc)) * 128
                            nc.tensor.matmul(
                                sp[:, j, qoff:DG],
                                lhsT=qkT[:, 4, kt * 128:(kt + 1) * 128],
                                rhs=qkT[:, h, qc * DG + qoff:(qc + 1) * DG],
                                start=True, stop=True)
                        pt = ptp.tile([128, 2, DG], bf16, tag="pt")
                        if 2 * j2 + 1 < 4 * qc:     # fully below diagonal
                            nc.scalar.activation(
                                out=pt.rearrange("p a b -> p (a b)"),
                                in_=sp.rearrange("p a b -> p (a b)"),
                                func=Exp, scale=ISD)
                        else:
                            for j in range(2):
                                kt = 2 * j2 + j
                                qoff = max(0, (kt - 4 * qc)) * 128
                                nc.scalar.activation(
                                    out=pt[:, j, qoff:DG], in_=sp[:, j, qoff:DG],
                                    func=Exp, scale=ISD)
                        pts.append(pt)
                    # causal masking of the 4 diagonal k tiles
                    for qtl in range(4):
                        kt = 4 * qc + qtl
                        blk = pts[kt // 2][:, kt % 2, qtl * 128:(qtl + 1) * 128]
                        nc.vector.tensor_mul(blk, blk, tri_sb)
                    # probs @ [v | ones] per q tile -> [q, d | denom]
                    for qtl in range(4):
                        qt = 4 * qc + qtl
                        op = pX.tile([128, DG], f32, tag="X")
                        for kt in range(qt + 1):
                            nc.tensor.matmul(
                                op[:, 0:129],
                                lhsT=pts[kt // 2][:, kt % 2, qtl * 128:(qtl + 1) * 128],
                                rhs=vvb[:, kt, 0:129],
                                start=(kt == 0), stop=(kt == qt))
                        rc = smp.tile([128, 1], f32, tag="rc")
                        nc.vector.reciprocal(rc, op[:, 128:129])
                        nc.vector.tensor_scalar_mul(
                            on_tiles[qtl][:, h, :], op[:, 0:128], rc)
                # batched xbar transposes into otile [d, h, q-chunk]
                otile = otp.tile([128, HG, DG], bf16, tag="ot")
                for qtl in range(4):
                    nc.sync.dma_start_transpose(
                        otile[:, :, qtl * 128:(qtl + 1) * 128], on_tiles[qtl])
                # output projection for this chunk's 4 row tiles
                for stl in range(4):
                    srow = (4 * qc + stl) * 128
                    for cch in range(2):
                        wop = pA.tile([128, 2, DG], f32, tag="A")
                        for cc2 in range(2):
                            for h2 in range(HG):
                                nc.tensor.matmul(
                                    wop[:, cc2, :],
                                    lhsT=otile[:, h2, stl * 128:(stl + 1) * 128],
                                    rhs=wo_sb[:, h2, (2 * cch + cc2) * DG:
                                              (2 * cch + cc2 + 1) * DG],
                                    start=(h2 == 0), stop=(h2 == HG - 1))
                        oc = ocp.tile([128, 2 * DG], bf16, tag="oc")
                        if cch == 0:
                            nc.vector.tensor_copy(
                                oc.rearrange("p (a b) -> p a b", a=2), wop)
                        else:
                            nc.scalar.copy(
                                oc.rearrange("p (a b) -> p a b", a=2), wop)
                        nc.sync.dma_start(
                            out[srow:srow + 128,
                                cch * 2 * DG:(cch + 1) * 2 * DG], oc)

            for st in range(PH1_TILES):
                emit_st(st)
            for qc in range(PH2_CHUNKS):
                emit_qc(qc)


def _build():
    nc = bacc.Bacc("TRN2", target_bir_lowering=False, debug=False,
                   num_devices=NCORES)
    xT = nc.dram_tensor("xT", [HID, S], bf16, kind="ExternalInput").ap()
    wqT = nc.dram_tensor("wqT", [HID, DG], bf16, kind="ExternalInput").ap()
    wkvT = nc.dram_tensor("wkvT", [HID, 2 * D], bf16, kind="ExternalInput").ap()
    woT = nc.dram_tensor("woT", [DG, HID], bf16, kind="ExternalInput").ap()
    csx = nc.dram_tensor("csx", [S, 128], bf16, kind="ExternalInput").ap()
    snx = nc.dram_tensor("snx", [S, 128], bf16, kind="ExternalInput").ap()
    tri = nc.dram_tensor("tri", [128, 128], bf16, kind="ExternalInput").ap()
    out = nc.dram_tensor("out", [S, HID], bf16, kind="ExternalOutput").ap()
    _emit(nc, xT, wqT, wkvT, woT, csx, snx, tri, out)
    nc.compile()
    return nc


def _get_compiled():
    global _compiled
    if _compiled is None:
        _compiled = _build()
    return _compiled


def kernel(x, cos, sin, wq, wk, wv, wo):
    nc = _get_compiled()
    x = np.asarray(x, np.float32)
    cos = np.asarray(cos, np.float32)
    sin = np.asarray(sin, np.float32)
    wq = np.asarray(wq, np.float32)
    wk = np.asarray(wk, np.float32)
    wv = np.asarray(wv, np.float32)
    wo = np.asarray(wo, np.float32)

    bf = ml_dtypes.bfloat16
    wkvT = np.ascontiguousarray(np.concatenate([wk, wv], 0).T.astype(bf))
    csx = np.ascontiguousarray(np.concatenate([cos, cos], 1).astype(bf))
    snx = np.ascontiguousarray(np.concatenate([sin, -sin], 1).astype(bf))
    tri = np.ascontiguousarray(np.triu(np.ones((128, 128), np.float32)).astype(bf))
    xTs = [np.ascontiguousarray(x[b].T.astype(bf)) for b in range(B)]
    wqTs = [np.ascontiguousarray(wq[g * DG:(g + 1) * DG].T.astype(bf))
            for g in range(GROUPS)]
    woTs = [np.ascontiguousarray(wo[:, g * DG:(g + 1) * DG].T.astype(bf))
            for g in range(GROUPS)]

    in_maps = []
    for c in range(NCORES):
        b, g = divmod(c, GROUPS)
        in_maps.append({
            "xT": xTs[b], "wqT": wqTs[g], "wkvT": wkvT, "woT": woTs[g],
            "csx": csx, "snx": snx, "tri": tri,
        })
    res = run_bass_kernel_spmd(nc, in_maps, list(range(NCORES)), trace=TRACE)
    LAST["res"] = res
    outs = [r["out"].astype(np.float32) for r in res.results]
    final = np.empty((B, S, HID), np.float32)
    for b in range(B):
        final[b] = (outs[GROUPS * b] + outs[GROUPS * b + 1]
                    + outs[GROUPS * b + 2] + outs[GROUPS * b + 3])
    return final
